# revision 1
# baseline (speedup 1.0000x reference)
"""Softsign multi-head attention on 8 Trainium2 NeuronCores (Bass/Tile).

Sharding: core c = 2*b + hg -> batch b (of 4), head-group hg (8 of 16 heads).
Data-parallel over batch, tensor-parallel over heads; the two half outputs
per batch are summed on host (out = ctx_lo @ Wo_lo.T + ctx_hi @ Wo_hi.T).

All matmuls run in float32r (TF32-like precision, full PE rate).
Softsign(s) = s/(1+|s|) is computed with ScalarE Abs (fused into the PSUM
evacuation) + ONE custom fused DVE op per tile:
  u = |s|+1; nu = bitcast(~u); y ~= 1/u via one minimax-Newton step;
  attn = s*y   (max rel err ~1.7e-3).
"""

import sys

sys.path.insert(0, "/opt/trn_rl_repo")

import numpy as np

import concourse.bass as bass
import concourse.dve_ops as dve_ops
import concourse.mybir as mybir
import concourse.tile as tile
from concourse.bass_utils import run_bass_kernel_spmd
from concourse.dve_ops import DveOp
from concourse.dve_spec import AluOp, Bin, One, Spec, Src0, Src1, lower
from concourse.dve_uop import DveOpSpec

f32 = mybir.dt.float32
f32r = mybir.dt.float32r
bf16 = mybir.dt.bfloat16
ATTN_DT = bf16  # dtype for q/k/v/attn (scores + ctx matmuls)
AF = mybir.ActivationFunctionType

# ---------------------------------------------------------------- softsign op
A_CONST = -0.4714038456062873
B_CONST = 0.055459279842660344


def _ref_fused_softsign(in0, in1, s0, s1, imm2):
    a = in0.astype(np.float32)
    s = in1.astype(np.float32)
    u = (a + np.float32(1.0)).astype(np.float32)
    nu = (~u.view(np.int32)).view(np.float32)
    W = (u * nu).astype(np.float32)
    r1 = (W * np.float32(s1)).astype(np.float32)
    w2 = (np.float32(s0) - r1).astype(np.float32)
    y1 = (nu * w2).astype(np.float32)
    return (s * y1).astype(np.float32)


def _register_softsign() -> DveOp:
    for existing in dve_ops.OPS:
        if existing.name == "SOFTSIGN_FIN_ANT":
            return existing
    u = Src0 + One
    nu = Bin(AluOp.BITWISE_NOT, u, u)
    W = u * nu
    from concourse.dve_spec import C0, C1

    body = Src1 * (nu * (C0 - W * C1))
    spec = Spec(body=body, reference=_ref_fused_softsign)
    shas = {}
    for ver in ("v3", "v4"):
        uops = lower(spec, ver=ver)
        tmp = DveOpSpec(name="SOFTSIGN_FIN_ANT", opcode=31, uops=uops, rd1_en=True)
        shas[ver] = tmp.sha(ver)
    op = DveOp("SOFTSIGN_FIN_ANT", spec, subdim=False, uops_sha=shas)
    dve_ops.OPS.append(op)
    dve_ops.CUSTOM_DVE_SPECS[op.name] = op.spec
    dve_ops._SUB_OPCODE_FOR_NAME[op.name] = (
        dve_ops._CUSTOM_DVE_ROW_BASE + len(dve_ops.OPS) - 1
    )
    return op


def _emit_softsign(nc, out, a, s):
    op = _register_softsign()
    return nc.vector._custom_dve(op, out=out, in0=a, in1=s, s0=A_CONST, s1=B_CONST)


# ------------------------------------------------------------- wait splitting
_ws_ctr = [0]


def _split_excess_waits(nc, limit=1):
    """This container's walrus accepts a single sync-wait command per
    instruction; push excess waits onto prefix NoOps on the same engine."""
    for f in nc.m.functions:
        for b in f.blocks:
            new_insts = []
            for inst in b.instructions:
                si = getattr(inst, "sync_info", None)
                ow = list(si.on_wait) if si and si.on_wait else []
                if len(ow) > limit:
                    excess, keep = ow[:-limit], ow[-limit:]
                    for i in range(0, len(excess), limit):
                        chunk = excess[i : i + limit]
                        _ws_ctr[0] += 1
                        nop = mybir.InstNoOp(
                            name=f"waitsplit-{_ws_ctr[0]}",
                            ins=[],
                            outs=[],
                            engine=inst.engine,
                            sync_info=mybir.SyncInfo(on_wait=chunk, on_update=[]),
                            text_hint="waitsplit",
                        )
                        nc.register_instruction(nop, overwrite=True)
                        new_insts.append(nop)
                    si.on_wait = keep
                new_insts.append(inst)
            b.instructions = new_insts


# --------------------------------------------------------------- kernel build
S, E, F, D = 2048, 1024, 512, 64
NE, NF, NS, NST, GRP = 8, 4, 4, 16, 2


def _build(reps=1):
    _register_softsign()
    nc = bass.Bass()
    xT_d = nc.declare_dram_parameter("xT", [E, S], f32r, isOutput=False)
    WqT_d = nc.declare_dram_parameter("WqT", [E, F], f32r, isOutput=False)
    WkT_d = nc.declare_dram_parameter("WkT", [E, F], f32r, isOutput=False)
    WvT_d = nc.declare_dram_parameter("WvT", [E, F], f32r, isOutput=False)
    WoT_d = nc.declare_dram_parameter("WoT", [F, E], f32r, isOutput=False)
    bq_d = nc.declare_dram_parameter("bq", [128, NF], f32, isOutput=False)
    bk_d = nc.declare_dram_parameter("bk", [128, NF], f32, isOutput=False)
    bv_d = nc.declare_dram_parameter("bv", [1, F], f32r, isOutput=False)
    bo_d = nc.declare_dram_parameter("bo", [1, E], f32r, isOutput=False)
    ones_d = nc.declare_dram_parameter("ones", [1, 128], f32r, isOutput=False)
    out_d = nc.declare_dram_parameter("out", [S, E], f32, isOutput=True)

    with tile.TileContext(nc) as tc:
        with tc.tile_pool(name="persist", bufs=1) as pp:
            q_sb = [pp.tile([128, S], ATTN_DT, tag=f"q{t}", name=f"q{t}")
                    for t in range(NF)]
            k_sb = [pp.tile([128, S], ATTN_DT, tag=f"k{t}", name=f"k{t}")
                    for t in range(NF)]
            v_sb = [pp.tile([128, F], ATTN_DT, tag=f"v{t}", name=f"v{t}")
                    for t in range(NST)]
            bq_sb = pp.tile([128, NF], f32, tag="bq")
            bk_sb = pp.tile([128, NF], f32, tag="bk")
            bv_sb = pp.tile([1, F], f32r, tag="bv")
            bo_sb = pp.tile([1, E], f32r, tag="bo")
            ones_sb = pp.tile([1, 128], f32r, tag="ones")
            nc.sync.dma_start(bq_sb[:], bq_d[:])
            nc.sync.dma_start(bk_sb[:], bk_d[:])
            nc.sync.dma_start(bv_sb[:], bv_d[:])
            nc.sync.dma_start(bo_sb[:], bo_d[:])
            nc.sync.dma_start(ones_sb[:], ones_d[:])

            # ---------------- Phase 1: q/k/v projections ----------------
            for _rep in range(reps):
              with (
                  tc.tile_pool(name=f"p1_{_rep}", bufs=1) as p1,
                  tc.tile_pool(name=f"psA{_rep}", bufs=2, space="PSUM") as psA,
                  tc.tile_pool(name=f"psB{_rep}", bufs=2, space="PSUM") as psB,
              ):
                  wq = [p1.tile([128, F], f32r, tag=f"wq{e}", name=f"wq{e}")
                        for e in range(NE)]
                  wk = [p1.tile([128, F], f32r, tag=f"wk{e}", name=f"wk{e}")
                        for e in range(NE)]
                  wv = [p1.tile([128, F], f32r, tag=f"wv{e}", name=f"wv{e}")
                        for e in range(NE)]
                  for e in range(NE):
                      nc.sync.dma_start(wq[e][:], WqT_d[e * 128:(e + 1) * 128, :])
                      nc.sync.dma_start(wk[e][:], WkT_d[e * 128:(e + 1) * 128, :])
                      nc.sync.dma_start(wv[e][:], WvT_d[e * 128:(e + 1) * 128, :])
                  with tc.tile_pool(name=f"xp{_rep}", bufs=2) as xp:
                      for ss in range(NS):
                          sl = slice(ss * 512, (ss + 1) * 512)
                          xt = []
                          for e in range(NE):
                              t = xp.tile([128, 512], f32r, tag=f"x{e}",
                                          name=f"x{e}")
                              nc.sync.dma_start(t[:],
                                                xT_d[e * 128:(e + 1) * 128, sl])
                              xt.append(t)
                          for w, dst, b_sb in ((wq, q_sb, bq_sb),
                                               (wk, k_sb, bk_sb)):
                              for ft in range(NF):
                                  ps = psA.tile([128, 512], f32, tag="proj",
                                                name="psproj")
                                  for e in range(NE):
                                      nc.tensor.matmul(
                                          ps[:],
                                          w[e][:, ft * 128:(ft + 1) * 128],
                                          xt[e][:],
                                          start=(e == 0), stop=(e == NE - 1),
                                          skip_group_check=(e > 0),
                                      )
                                  nc.scalar.activation(
                                      dst[ft][:, sl], ps[:], AF.Identity,
                                      bias=b_sb[:, ft:ft + 1],
                                  )
                          for st4 in range(4):
                              st = ss * 4 + st4
                              ps = psB.tile([128, 512], f32, tag="vproj",
                                            name="psv")
                              for e in range(NE):
                                  nc.tensor.matmul(
                                      ps[:],
                                      xt[e][:, st4 * 128:(st4 + 1) * 128],
                                      wv[e][:],
                                      start=(e == 0), stop=False,
                                      skip_group_check=(e > 0),
                                  )
                              nc.tensor.matmul(
                                  ps[:], ones_sb[:], bv_sb[:],
                                  start=False, stop=True, skip_group_check=True,
                              )
                              nc.scalar.copy(v_sb[st][:], ps[:])

              # ------------- Phase 2+3: attention + out-projection ---------
              with (
                  tc.tile_pool(name=f"p2{_rep}", bufs=1) as p2,
                  tc.tile_pool(name=f"pscore{_rep}", bufs=1, space="PSUM") as pscore,
                  tc.tile_pool(name=f"pctx{_rep}", bufs=1, space="PSUM") as pctx,
                  tc.tile_pool(name=f"pout{_rep}", bufs=2, space="PSUM") as pout,
                  tc.tile_pool(name=f"ap{_rep}", bufs=4) as ap_pool,
                  tc.tile_pool(name=f"atp{_rep}", bufs=4) as at_pool,
                  tc.tile_pool(name=f"op{_rep}", bufs=4) as o_pool,
              ):
                  ctx_sb = [p2.tile([128, S], f32r, tag=f"c{t}", name=f"c{t}")
                            for t in range(NF)]
                  wo = [p2.tile([128, E], f32r, tag=f"wo{t}", name=f"wo{t}")
                        for t in range(NF)]
                  for t in range(NF):
                      nc.sync.dma_start(wo[t][:], WoT_d[t * 128:(t + 1) * 128, :])

                  for ss in range(NS):
                      sl = slice(ss * 512, (ss + 1) * 512)
                      for hp in range(NF):
                          psc = [pctx.tile([64, 512], f32, tag=f"ctx{p}",
                                           name=f"psctx{p}") for p in range(2)]
                          for g in range(NST // GRP):
                              pss_p, at_p = [], []
                              for p in range(2):
                                  rows = slice(p * 64, (p + 1) * 64)
                                  pss = pscore.tile([128, 512 * GRP], f32,
                                                    tag=f"score{p}",
                                                    name=f"psscore{p}")
                                  for jj in range(GRP):
                                      j = GRP * g + jj
                                      nc.tensor.matmul(
                                          pss[:, jj * 512:(jj + 1) * 512],
                                          k_sb[hp][rows, j * 128:(j + 1) * 128],
                                          q_sb[hp][rows, sl],
                                          start=True, stop=True,
                                      )
                                  pss_p.append(pss)
                              for p in range(2):
                                  a_t = ap_pool.tile([128, 512 * GRP], f32,
                                                     tag=f"abs{p}",
                                                     name=f"absT{p}")
                                  nc.scalar.activation(a_t[:], pss_p[p][:],
                                                       AF.Abs)
                                  at_t = at_pool.tile([128, 512 * GRP], ATTN_DT,
                                                      tag=f"attn{p}",
                                                      name=f"attnT{p}")
                                  _emit_softsign(nc, at_t[:], a_t[:], pss_p[p][:])
                                  at_p.append(at_t)
                              for jj in range(GRP):
                                  j = GRP * g + jj
                                  for p in range(2):
                                      h = 2 * hp + p
                                      nc.tensor.matmul(
                                          psc[p][:],
                                          v_sb[j][:, h * 64:(h + 1) * 64],
                                          at_p[p][:, jj * 512:(jj + 1) * 512],
                                          start=(g == 0 and jj == 0),
                                          stop=(g == NST // GRP - 1
                                                and jj == GRP - 1),
                                          skip_group_check=not (g == 0
                                                                and jj == 0),
                                      )
                          for p in range(2):
                              rows = slice(p * 64, (p + 1) * 64)
                              nc.scalar.copy(ctx_sb[hp][rows, sl], psc[p][:])
                      for st4 in range(4):
                          st = ss * 4 + st4
                          for eh in range(2):
                              esl = slice(eh * 512, (eh + 1) * 512)
                              pso = pout.tile([128, 512], f32, tag="out",
                                              name="psout")
                              for hp in range(NF):
                                  nc.tensor.matmul(
                                      pso[:],
                                      ctx_sb[hp][:, ss * 512 + st4 * 128:
                                                 ss * 512 + (st4 + 1) * 128],
                                      wo[hp][:, esl],
                                      start=(hp == 0), stop=False,
                                      skip_group_check=(hp > 0),
                                  )
                              nc.tensor.matmul(
                                  pso[:], ones_sb[:], bo_sb[:, esl],
                                  start=False, stop=True, skip_group_check=True,
                              )
                              o_t = o_pool.tile([128, 512], f32, tag="ot",
                                                name="otile")
                              nc.scalar.copy(o_t[:], pso[:])
                              nc.sync.dma_start(
                                  out_d[st * 128:(st + 1) * 128, esl], o_t[:]
                              )

    mybir.codegen_inst_isa_subclasses(nc)
    _split_excess_waits(nc, 1)
    return nc


_NC_CACHE = None


def _get_nc():
    global _NC_CACHE
    if _NC_CACHE is None:
        _NC_CACHE = _build()
    return _NC_CACHE


def make_in_maps(x, Wq, bq, Wk, bk, Wv, bv, Wo, bo):
    """Per-core input dicts for cores 0..7 (core = 2*b + hg)."""
    x = np.asarray(x, np.float32)
    in_maps = []
    ones = np.ones((1, 128), np.float32)
    for c in range(8):
        b, hg = divmod(c, 2)
        fs = slice(hg * 512, (hg + 1) * 512)
        in_maps.append({
            "xT": np.ascontiguousarray(x[b].T),
            "WqT": np.ascontiguousarray((np.asarray(Wq)[fs, :] / 8.0).T.astype(np.float32)),
            "WkT": np.ascontiguousarray(np.asarray(Wk)[fs, :].T.astype(np.float32)),
            "WvT": np.ascontiguousarray(np.asarray(Wv)[fs, :].T.astype(np.float32)),
            "WoT": np.ascontiguousarray(np.asarray(Wo)[:, fs].T.astype(np.float32)),
            "bq": np.ascontiguousarray(
                (np.asarray(bq)[fs] / 8.0).astype(np.float32).reshape(4, 128).T),
            "bk": np.ascontiguousarray(
                np.asarray(bk)[fs].astype(np.float32).reshape(4, 128).T),
            "bv": np.asarray(bv)[fs].astype(np.float32).reshape(1, 512),
            "bo": (np.asarray(bo).astype(np.float32) / 2.0).reshape(1, 1024),
            "ones": ones,
        })
    return in_maps


def kernel(x, Wq, bq, Wk, bk, Wv, bv, Wo, bo):
    nc = _get_nc()
    in_maps = make_in_maps(x, Wq, bq, Wk, bk, Wv, bv, Wo, bo)
    res = run_bass_kernel_spmd(nc, in_maps, list(range(8))).results
    out = np.empty((4, S, E), np.float32)
    for b in range(4):
        out[b] = res[2 * b]["out"] + res[2 * b + 1]["out"]
    return (out,)


if __name__ == "__main__":
    rng = np.random.RandomState(0)
    s = 1.0 / np.sqrt(E)
    inputs = dict(
        x=rng.randn(4, S, E).astype(np.float32),
        Wq=rng.uniform(-s, s, (E, E)).astype(np.float32),
        bq=rng.uniform(-s, s, E).astype(np.float32),
        Wk=rng.uniform(-s, s, (E, E)).astype(np.float32),
        bk=rng.uniform(-s, s, E).astype(np.float32),
        Wv=rng.uniform(-s, s, (E, E)).astype(np.float32),
        bv=rng.uniform(-s, s, E).astype(np.float32),
        Wo=rng.uniform(-s, s, (E, E)).astype(np.float32),
        bo=rng.uniform(-s, s, E).astype(np.float32),
    )
    out = kernel(**inputs)[0]
    print("out", out.shape, out.dtype, float(np.abs(out).max()))

